# revision 2
# baseline (speedup 1.0000x reference)
"""GRU scan kernel for Trainium2, 8-core data-parallel.

Problem: B=64, S=512, I=512, H=1024, O=2 GRU + FC + log_softmax.
Strategy: shard batch 8-way (8 rows/core). Per core:
  1. Precompute xpart[b,t,:] = x[b,t,:] @ Wx_all + b_all  (fp32r matmuls,
     [r|z|hc] column order), store bf16 in internal DRAM.
  2. 512-step scan. Per step, recurrent matmuls in bf16 with a packed
     layout: preacts computed as chunks of [8 batch, 256 feat] placed at
     psum partitions 32j so four PE column-groups stream concurrently
     (the PE moving-operand port is the bottleneck at 1 elem/port/cycle).
     xpart is prefetched 8 steps ahead in the psum-scattered layout and
     injected by Act/DVE psum-bank initialization (start=False matmuls
     accumulate on top), keeping the PE stream pure weight traffic.
     h^T (the matmul stationary operand) is maintained via a PE
     transpose of the packed h each step.
  3. FC head + log_softmax on device.
"""

import os
import sys
from contextlib import ExitStack

for _p in ("/opt/trn_rl_repo",):
    if os.path.isdir(_p) and _p not in sys.path:
        sys.path.insert(0, _p)

import numpy as np
import ml_dtypes

import concourse.bass as bass
import concourse.mybir as mybir
import concourse.tile as tile
from concourse import bacc
from concourse.bass import ds
from concourse.bass_utils import run_bass_kernel_spmd

B, S, I, H, O = 64, 512, 512, 1024, 2
NCORES = 8
BL = B // NCORES          # 8 batch rows per core
G3 = 3 * H                # 3072 gate features, order [r | z | hc]
KT = H // 128             # 8 k-tiles over hidden dim
F32, F32R, BF16 = mybir.dt.float32, mybir.dt.float32r, mybir.dt.bfloat16
AFT = mybir.ActivationFunctionType


def _lhsT(q, k, w=8):
    """[128, w] stationary slice for hidden k-tile k from a Q-layout tile.

    Q layout [128, 2, 128]: entry [p, u, 32j+b] = v[b, 256j + 128u + p].
    k-tile k (hidden feats [128k, 128k+128)) -> u = k%2, cols 32*(k//2)+[0,w).
    Columns 8..w hold other k-tiles' data; the junk output rows they produce
    keep the full psum block initialized.
    """
    j, u = divmod(k, 2)
    return q[:, u, 32 * j : 32 * j + w]


def _scan_step(nc, tc, pools, consts, xps, slot):
    """Emit one GRU step. xps[:, slot, g, :] holds the psum-scattered xpart."""
    (ptmp, ppsum, ptps) = pools
    (whh, id128, hQ, hQbf) = consts

    # ---- phase A: r and z preacts. chunk j covers gate feats [256j, 256j+256)
    # at psum partitions [32*(j%4), +8). r = chunks 0-3 (tile A), z = 4-7 (B).
    # Banks are pre-initialized with xpart by DVE copies (off the PE critical
    # path: they run while the previous step's matmuls stream) and all gate
    # matmuls accumulate with start=False.
    r_ps = ppsum.tile([128, 256], F32, tag="r_ps")
    nc.scalar.copy(r_ps, xps[:, slot, 0, :])
    z_ps = ppsum.tile([128, 256], F32, tag="z_ps")
    nc.scalar.copy(z_ps, xps[:, slot, 1, :])
    hc_ps = ppsum.tile([128, 256], F32, tag="hc_ps")
    nc.vector.tensor_copy(hc_ps, xps[:, slot, 2, :])

    def gate_mm(ps, j, k):
        b0 = 32 * (j % 4)
        out = ps[b0 : b0 + 32, :]
        nc.tensor.matmul(out, _lhsT(hQbf, k, 32),
                         whh[:, G3 * k + 256 * j : G3 * k + 256 * j + 256],
                         start=False, stop=(k == KT - 1),
                         tile_position=(0, b0), skip_group_check=True)

    # r chunks first (4-way), then z chunks (4-way) -> r-half finishes early
    for k in range(KT):
        for j in range(4):
            gate_mm(r_ps, j, k)
    for k in range(KT):
        for j in range(4, 8):
            gate_mm(z_ps, j, k)

    r_sb = ptmp.tile([128, 256], F32, tag="r_sb")
    nc.scalar.activation(r_sb, r_ps, AFT.Sigmoid)
    z_sb = ptmp.tile([128, 256], F32, tag="z_sb")
    nc.scalar.activation(z_sb, z_ps, AFT.Sigmoid)

    # transpose sigmoided preacts into Q space
    rQ = ptmp.tile([128, 2, 128], F32, tag="rQ")
    zQ = ptmp.tile([128, 2, 128], F32, tag="zQ")
    for u in range(2):
        tp = ptps.tile([128, 128], F32, tag="tp")
        nc.tensor.transpose(tp, r_sb[:, 128 * u : 128 * (u + 1)], id128)
        nc.vector.tensor_copy(rQ[:, u, :], tp)
    for u in range(2):
        tp = ptps.tile([128, 128], F32, tag="tp")
        nc.tensor.transpose(tp, z_sb[:, 128 * u : 128 * (u + 1)], id128)
        nc.scalar.copy(zQ[:, u, :], tp)

    # rh^T directly in Q space (bf16 out feeds the hc matmul)
    rhQ = ptmp.tile([128, 2, 128], BF16, tag="rhQ")
    nc.vector.tensor_mul(rhQ, rQ, hQ)

    # ---- phase B: hc preact. 4 chunks of 256 at the 4 bases (4-way).
    for k in range(KT):
        for j in range(4):
            b0 = 32 * j
            nc.tensor.matmul(
                hc_ps[b0 : b0 + 32, :], _lhsT(rhQ, k, 32),
                whh[:, G3 * k + 2048 + 256 * j : G3 * k + 2048 + 256 * j + 256],
                start=False, stop=(k == KT - 1),
                tile_position=(0, b0), skip_group_check=True)

    hc_sb = ptmp.tile([128, 256], F32, tag="hc_sb")
    nc.scalar.activation(hc_sb, hc_ps, AFT.Tanh)
    hcQ = ptmp.tile([128, 2, 128], F32, tag="hcQ")
    for u in range(2):
        tp = ptps.tile([128, 128], F32, tag="tp")
        nc.tensor.transpose(tp, hc_sb[:, 128 * u : 128 * (u + 1)], id128)
        nc.scalar.activation(hcQ[:, u, :], tp, AFT.Copy)

    # blend in Q space: h = h + z*(hc - h)
    d1 = ptmp.tile([128, 2, 128], F32, tag="d1")
    nc.vector.tensor_sub(d1, hcQ, hQ)
    d2 = ptmp.tile([128, 2, 128], F32, tag="d2")
    nc.vector.tensor_mul(d2, zQ, d1)
    nc.vector.tensor_add(hQ, hQ, d2)
    nc.vector.tensor_copy(hQbf, hQ)


def build(nc_s=S, unroll=8, use_for_i=True, scan_reps=1):
    """Build the Bass program. nc_s = number of scan steps (512 for real)."""
    assert nc_s % 128 == 0  # precompute scatter assumes whole-b row tiles
    nc = bacc.Bacc("TRN2", target_bir_lowering=False, debug=False,
                   num_devices=NCORES)

    xt_d = nc.dram_tensor("xt", [I, BL * nc_s], F32R, kind="ExternalInput")
    h0q_d = nc.dram_tensor("h0q", [128, 2, 128], F32, kind="ExternalInput")
    wx_d = nc.dram_tensor("wx", [I, G3], F32R, kind="ExternalInput")
    bias_d = nc.dram_tensor("bias", [1, G3], F32R, kind="ExternalInput")
    whh_d = nc.dram_tensor("whh", [H, G3], BF16, kind="ExternalInput")
    wfc_d = nc.dram_tensor("wfc", [H, O], F32R, kind="ExternalInput")
    bfc_d = nc.dram_tensor("bfc", [1, O], F32R, kind="ExternalInput")
    id128f_d = nc.dram_tensor("id128f", [128, 128], F32, kind="ExternalInput")
    ones_d = nc.dram_tensor("ones", [1, 128], F32R, kind="ExternalInput")
    out_d = nc.dram_tensor("out", [BL, O], F32, kind="ExternalOutput")

    # xpart[b, t, f] (t-padded so the prefetch can overrun past nc_s)
    SPAD = 16
    xpart_d = nc.dram_tensor("xpart", [BL, nc_s + SPAD, G3], BF16)

    n_rows = BL * nc_s
    n_rt = (n_rows + 127) // 128

    with tile.TileContext(nc) as tc, ExitStack() as ctx:
        # ---------------- constants resident in SBUF ----------------
        pconst = ctx.enter_context(tc.tile_pool(name="pconst", bufs=1))
        whh = pconst.tile([128, KT * G3], BF16)
        for k in range(KT):
            nc.sync.dma_start(out=whh[:, G3 * k : G3 * (k + 1)],
                              in_=whh_d[128 * k : 128 * (k + 1), :])
        wx = pconst.tile([128, 4 * G3], F32R)
        for k in range(4):
            nc.sync.dma_start(out=wx[:, G3 * k : G3 * (k + 1)],
                              in_=wx_d[128 * k : 128 * (k + 1), :])
        bias_sb = pconst.tile([1, G3], F32R)
        nc.sync.dma_start(out=bias_sb, in_=bias_d[:, :])
        id128 = pconst.tile([128, 128], F32)
        nc.sync.dma_start(out=id128, in_=id128f_d[:, :])
        ones128 = pconst.tile([1, 128], F32R)
        nc.sync.dma_start(out=ones128, in_=ones_d[:, :])
        wfc = pconst.tile([128, KT * O], F32R)
        for k in range(KT):
            nc.sync.dma_start(out=wfc[:, O * k : O * (k + 1)],
                              in_=wfc_d[128 * k : 128 * (k + 1), :])
        bfc_sb = pconst.tile([1, O], F32R)
        nc.sync.dma_start(out=bfc_sb, in_=bfc_d[:, :])

        # persistent scan state (Q layout)
        hQ = pconst.tile([128, 2, 128], F32)
        nc.sync.dma_start(out=hQ, in_=h0q_d[:, :, :])
        hQbf = pconst.tile([128, 2, 128], BF16)
        nc.vector.tensor_copy(hQbf, hQ)

        # xpart staging tiles (ping-pong, 8 steps each, psum-scattered layout)
        xpsA = pconst.tile([128, 8, 3, 256], BF16)
        xpsB = pconst.tile([128, 8, 3, 256], BF16)
        nc.vector.memset(xpsA, 0.0)
        nc.vector.memset(xpsB, 0.0)

        # ---------------- precompute xpart ----------------
        ppre = ctx.enter_context(tc.tile_pool(name="ppre", bufs=2))
        ppre_ps = ctx.enter_context(tc.tile_pool(name="ppre_ps", bufs=2,
                                                 space="PSUM"))
        # t-major order: all t0=0 tiles are stored first, so their DRAM
        # writes complete long before the scan's first prefetch reads them
        # (DMA->DMA RAW through DRAM is not tracked by the tile framework).
        n_tb = nc_s // 128
        for rt in [b_ * n_tb + tb for tb in range(n_tb) for b_ in range(BL)]:
            r0 = rt * 128
            b = r0 // nc_s
            t0 = r0 % nc_s
            xt_sb = ppre.tile([128, 4, 128], F32R, tag="xt_sb")
            for k in range(4):
                nc.sync.dma_start(
                    out=xt_sb[:, k, :],
                    in_=xt_d[128 * k : 128 * (k + 1), r0 : r0 + 128])
            xp_sb = ppre.tile([128, G3], BF16, tag="xp_sb")
            for n in range(6):
                xp_ps = ppre_ps.tile([128, 512], F32, tag="xp_ps")
                ns = slice(512 * n, 512 * (n + 1))
                nc.tensor.matmul(xp_ps, ones128, bias_sb[:, ns],
                                 start=True, stop=False)
                for k in range(4):
                    nc.tensor.matmul(
                        xp_ps, xt_sb[:, k, :],
                        wx[:, G3 * k + 512 * n : G3 * k + 512 * (n + 1)],
                        start=False, stop=(k == 3))
                if n % 2 == 1:
                    nc.scalar.copy(xp_sb[:, ns], xp_ps)
                else:
                    nc.vector.tensor_copy(xp_sb[:, ns], xp_ps)
            nc.sync.dma_start(out=xpart_d[b, t0 : t0 + 128, :], in_=xp_sb)

        # ---------------- scan ----------------
        ptmp = ctx.enter_context(tc.tile_pool(name="ptmp", bufs=1))
        ppsum = ctx.enter_context(tc.tile_pool(name="ppsum", bufs=1, space="PSUM"))
        ptps = ctx.enter_context(tc.tile_pool(name="ptps", bufs=3, space="PSUM"))
        pools = (ptmp, ppsum, ptps)
        consts = (whh, id128, hQ, hQbf)

        def prefetch(dst, t_ap):
            # scatter [b, t, f] -> psum layout [32a+b, slot, g, c] on the read
            for a in range(4):
                for g in range(3):
                    fs = 1024 * g + 256 * a
                    nc.sync.dma_start(
                        out=dst[32 * a : 32 * a + 8, :, g, :],
                        in_=xpart_d[0:BL, t_ap, fs : fs + 256])

        prefetch(xpsA, ds(0, 8))
        with tc.For_i(0, nc_s, 16) as iv:
            prefetch(xpsB, ds(iv + 8, 8))
            for u in range(8):
                _scan_step(nc, tc, pools, consts, xpsA, u)
            prefetch(xpsA, ds(iv + 16, 8))
            for u in range(8):
                _scan_step(nc, tc, pools, consts, xpsB, u)

        # ---------------- FC head + log_softmax ----------------
        hrelu = ptmp.tile([128, 2, 128], F32R, tag="hrelu")
        nc.scalar.activation(hrelu, hQ, AFT.Relu)

        fc_ps = ptps.tile([BL, O], F32, tag="tp")
        nc.tensor.matmul(fc_ps, ones128[:, :BL], bfc_sb, start=True, stop=False)
        for k in range(KT):
            nc.tensor.matmul(fc_ps, _lhsT(hrelu, k),
                             wfc[:, O * k : O * (k + 1)],
                             start=False, stop=(k == KT - 1))

        mx = ptmp.tile([BL, 1], F32, tag="mx")
        nc.vector.tensor_reduce(mx, fc_ps, mybir.AxisListType.X,
                                mybir.AluOpType.max)
        tt = ptmp.tile([BL, O], F32, tag="tt")
        nc.vector.tensor_scalar(tt, fc_ps, mx, None, mybir.AluOpType.subtract)
        ex = ptmp.tile([BL, O], F32, tag="ex")
        nc.scalar.activation(ex, tt, AFT.Exp)
        sm = ptmp.tile([BL, 1], F32, tag="sm")
        nc.vector.tensor_reduce(sm, ex, mybir.AxisListType.X,
                                mybir.AluOpType.add)
        lsm = ptmp.tile([BL, 1], F32, tag="lsm")
        nc.scalar.activation(lsm, sm, AFT.Ln)
        res = ptmp.tile([BL, O], F32, tag="res")
        nc.vector.tensor_scalar(res, tt, lsm, None, mybir.AluOpType.subtract)
        nc.sync.dma_start(out=out_d[:, :], in_=res)

    nc.compile()
    return nc


def prep_inputs(x, h, Wz, bz, Wr, br, Wh, bh, Wfc, bfc, nc_s=S):
    """Host-side prep: shard + relayout. Returns per-core input maps."""
    f32 = np.float32
    x = np.asarray(x, f32)[:, :nc_s, :]
    h0 = np.asarray(h, f32)[:, 0, :]
    Wx_all = np.concatenate([np.asarray(Wr, f32)[:I], np.asarray(Wz, f32)[:I],
                             np.asarray(Wh, f32)[:I]], axis=1)
    b_all = np.concatenate([np.asarray(br, f32), np.asarray(bz, f32),
                            np.asarray(bh, f32)])[None, :]
    Whh_all = np.concatenate([np.asarray(Wr, f32)[I:], np.asarray(Wz, f32)[I:],
                              np.asarray(Wh, f32)[I:]], axis=1)
    Whh_bf = Whh_all.astype(ml_dtypes.bfloat16)
    id128 = np.eye(128, dtype=f32)
    id8 = np.zeros((8, 32), ml_dtypes.bfloat16)
    np.fill_diagonal(id8[:, :8], 1)
    wfc = np.asarray(Wfc, f32)
    bfc_a = np.asarray(bfc, f32)[None, :]

    in_maps = []
    for c in range(NCORES):
        xc = x[c * BL : (c + 1) * BL]                      # [8, S, I]
        xt = xc.reshape(BL * nc_s, I).T.copy()             # [I, 8*S]
        h0c = h0[c * BL : (c + 1) * BL]                    # [8, H]
        # Q layout: h0q[p, u, 32j+b] = h0c[b, 256j + 128u + p]
        h0q = np.zeros((128, 2, 128), f32)
        hv = h0c.reshape(BL, 4, 2, 128)                    # [b, j, u, p]
        for j in range(4):
            h0q[:, :, 32 * j : 32 * j + BL] = hv[:, j].transpose(2, 1, 0)
        in_maps.append({
            "xt": xt, "h0q": h0q,
            "wx": Wx_all, "bias": b_all, "whh": Whh_bf,
            "wfc": wfc, "bfc": bfc_a,
            "id128f": id128, "id8": id8, "ones": np.ones((1, 128), f32),
        })
    return in_maps


_BUILT = {}
_LAST_RESULTS = None


def kernel(**inputs):
    global _LAST_RESULTS
    key = "full"
    if key not in _BUILT:
        _BUILT[key] = build(S, unroll=8, use_for_i=True)
    nc = _BUILT[key]
    in_maps = prep_inputs(**inputs)
    trace = bool(int(os.environ.get("BASS_TRACE", "0") or "0"))
    res = run_bass_kernel_spmd(nc, in_maps, list(range(NCORES)), trace=trace)
    _LAST_RESULTS = res
    outs = [res.results[c]["out"] for c in range(NCORES)]
    return np.concatenate(outs, axis=0).astype(np.float32)


if __name__ == "__main__":
    np.random.seed(0)
    print("building...")
    nc = build(128, unroll=8, use_for_i=True)
    print("build ok:", nc)



# revision 3
# speedup vs baseline: 1.0677x; 1.0677x over previous
"""GRU scan kernel for Trainium2, 8-core data-parallel.

Problem: B=64, S=512, I=512, H=1024, O=2 GRU + FC + log_softmax.
Strategy: shard batch 8-way (8 rows/core). Per core:
  1. Precompute xpart[b,t,:] = x[b,t,:] @ Wx_all + b_all  (fp32r matmuls,
     [r|z|hc] column order), store bf16 in internal DRAM.
  2. 512-step scan. Per step, recurrent matmuls in bf16 with a packed
     layout: preacts computed as 16 chunks of [8 batch, 128 feat] placed
     at psum partitions 8c so four PE column-groups run concurrently.
     Elementwise work runs densely packed on 128 (z,r) / 64 (hc)
     partitions. h^T (the matmul stationary operand) is maintained via a
     PE transpose of the packed h each step.
  3. FC head + log_softmax on device.
"""

import os
import sys
from contextlib import ExitStack

for _p in ("/opt/trn_rl_repo",):
    if os.path.isdir(_p) and _p not in sys.path:
        sys.path.insert(0, _p)

import numpy as np
import ml_dtypes

import concourse.bass as bass
import concourse.mybir as mybir
import concourse.tile as tile
from concourse import bacc
from concourse.bass import ds
from concourse.bass_utils import run_bass_kernel_spmd

B, S, I, H, O = 64, 512, 512, 1024, 2
NCORES = 8
BL = B // NCORES          # 8 batch rows per core
G3 = 3 * H                # 3072 gate features, order [r | z | hc]
KT = H // 128             # 8 k-tiles over hidden dim
F32, F32R, BF16 = mybir.dt.float32, mybir.dt.float32r, mybir.dt.bfloat16
AFT = mybir.ActivationFunctionType


def _lhsT(q, k, w=8):
    """[128, w] stationary slice for hidden k-tile k from a Q-layout tile.

    Q layout [128, 2, 128]: entry [p, u, 32j+b] = v[b, 256j + 128u + p].
    k-tile k (hidden feats [128k, 128k+128)) -> u = k%2, cols 32*(k//2)+[0,w).
    Columns 8..w hold other k-tiles' data; the junk output rows they produce
    keep the full psum block initialized.
    """
    j, u = divmod(k, 2)
    return q[:, u, 32 * j : 32 * j + w]


def _scan_step(nc, tc, pools, consts, xps, slot):
    """Emit one GRU step. xps[:, slot, g, :] holds the psum-scattered xpart."""
    (ptmp, ppsum, ptps) = pools
    (whh, id128, hQ, hQbf) = consts

    # ---- phase A: r and z preacts. chunk j covers gate feats [256j, 256j+256)
    # at psum partitions [32*(j%4), +8). r = chunks 0-3 (tile A), z = 4-7 (B).
    # Banks are pre-initialized with xpart by DVE copies (off the PE critical
    # path: they run while the previous step's matmuls stream) and all gate
    # matmuls accumulate with start=False.
    r_ps = ppsum.tile([128, 256], F32, tag="r_ps")
    nc.scalar.copy(r_ps, xps[:, slot, 0, :])
    z_ps = ppsum.tile([128, 256], F32, tag="z_ps")
    nc.scalar.copy(z_ps, xps[:, slot, 1, :])
    hc_ps = ppsum.tile([128, 256], F32, tag="hc_ps")
    nc.vector.tensor_copy(hc_ps, xps[:, slot, 2, :])

    def gate_mm(ps, j, k):
        b0 = 32 * (j % 4)
        out = ps[b0 : b0 + 32, :]
        nc.tensor.matmul(out, _lhsT(hQbf, k, 32),
                         whh[:, G3 * k + 256 * j : G3 * k + 256 * j + 256],
                         start=False, stop=(k == KT - 1),
                         tile_position=(0, b0), skip_group_check=True)

    # r chunks first (4-way), then z chunks (4-way) -> r-half finishes early
    for k in range(KT):
        for j in range(4):
            gate_mm(r_ps, j, k)
    for k in range(KT):
        for j in range(4, 8):
            gate_mm(z_ps, j, k)

    r_sb = ptmp.tile([128, 256], F32, tag="r_sb")
    nc.scalar.activation(r_sb, r_ps, AFT.Sigmoid)
    z_sb = ptmp.tile([128, 256], F32, tag="z_sb")
    nc.scalar.activation(z_sb, z_ps, AFT.Sigmoid)

    # rh^T fused straight off the transpose (per u half) so the hc matmuls
    # for even k-tiles (which read u=0) can start while the u=1 half lands.
    rhQ = ptmp.tile([128, 2, 128], BF16, tag="rhQ")
    for u in range(2):
        tp = ptps.tile([128, 128], F32, tag="tp")
        nc.tensor.transpose(tp, r_sb[:, 128 * u : 128 * (u + 1)], id128)
        nc.vector.tensor_mul(rhQ[:, u, :], tp, hQ[:, u, :])

    # ---- phase B: hc preact. Even k-tiles first (need only rhQ u=0).
    for k in (0, 2, 4, 6, 1, 3, 5, 7):
        for j in range(4):
            b0 = 32 * j
            nc.tensor.matmul(
                hc_ps[b0 : b0 + 32, :], _lhsT(rhQ, k, 32),
                whh[:, G3 * k + 2048 + 256 * j : G3 * k + 2048 + 256 * j + 256],
                start=False, stop=(k == KT - 1),
                tile_position=(0, b0), skip_group_check=True)

    # z transposes after the hc matmuls: zQ isn't needed until the blend,
    # and this keeps the PE from stalling on sigmoid(z) before hc.
    zQ = ptmp.tile([128, 2, 128], F32, tag="zQ")
    for u in range(2):
        tp = ptps.tile([128, 128], F32, tag="tp")
        nc.tensor.transpose(tp, z_sb[:, 128 * u : 128 * (u + 1)], id128)
        nc.scalar.copy(zQ[:, u, :], tp)

    hc_sb = ptmp.tile([128, 256], F32, tag="hc_sb")
    nc.scalar.activation(hc_sb, hc_ps, AFT.Tanh)
    hcQ = ptmp.tile([128, 2, 128], F32, tag="hcQ")
    for u in range(2):
        tp = ptps.tile([128, 128], F32, tag="tp")
        nc.tensor.transpose(tp, hc_sb[:, 128 * u : 128 * (u + 1)], id128)
        nc.scalar.activation(hcQ[:, u, :], tp, AFT.Copy)

    # blend in Q space, pipelined in 32-col j-slices: h = h + z*(hc - h).
    # Next step's r/z matmuls consume hQbf slice j for k-tiles 2j, 2j+1,
    # so they start as soon as slice 0 lands instead of after the full tile.
    d1 = ptmp.tile([128, 2, 128], F32, tag="d1")
    d2 = ptmp.tile([128, 2, 128], F32, tag="d2")
    for j4 in range(4):
        sl = slice(32 * j4, 32 * j4 + 32)
        nc.vector.tensor_sub(d1[:, :, sl], hcQ[:, :, sl], hQ[:, :, sl])
        nc.vector.tensor_mul(d2[:, :, sl], zQ[:, :, sl], d1[:, :, sl])
        nc.vector.tensor_add(hQ[:, :, sl], hQ[:, :, sl], d2[:, :, sl])
        nc.vector.tensor_copy(hQbf[:, :, sl], hQ[:, :, sl])


def build(nc_s=S, unroll=8, use_for_i=True, scan_reps=1):
    """Build the Bass program. nc_s = number of scan steps (512 for real)."""
    assert nc_s % 128 == 0  # precompute scatter assumes whole-b row tiles
    nc = bacc.Bacc("TRN2", target_bir_lowering=False, debug=False,
                   num_devices=NCORES)

    xt_d = nc.dram_tensor("xt", [I, BL * nc_s], F32R, kind="ExternalInput")
    h0q_d = nc.dram_tensor("h0q", [128, 2, 128], F32, kind="ExternalInput")
    wx_d = nc.dram_tensor("wx", [I, G3], F32R, kind="ExternalInput")
    bias_d = nc.dram_tensor("bias", [1, G3], F32R, kind="ExternalInput")
    whh_d = nc.dram_tensor("whh", [H, G3], BF16, kind="ExternalInput")
    wfc_d = nc.dram_tensor("wfc", [H, O], F32R, kind="ExternalInput")
    bfc_d = nc.dram_tensor("bfc", [1, O], F32R, kind="ExternalInput")
    id128f_d = nc.dram_tensor("id128f", [128, 128], F32, kind="ExternalInput")
    ones_d = nc.dram_tensor("ones", [1, 128], F32R, kind="ExternalInput")
    out_d = nc.dram_tensor("out", [BL, O], F32, kind="ExternalOutput")

    # xpart[b, t, f] (t-padded so the prefetch can overrun past nc_s)
    SPAD = 16
    xpart_d = nc.dram_tensor("xpart", [BL, nc_s + SPAD, G3], BF16)

    n_rows = BL * nc_s
    n_rt = (n_rows + 127) // 128

    with tile.TileContext(nc) as tc, ExitStack() as ctx:
        # ---------------- constants resident in SBUF ----------------
        pconst = ctx.enter_context(tc.tile_pool(name="pconst", bufs=1))
        whh = pconst.tile([128, KT * G3], BF16)
        for k in range(KT):
            nc.sync.dma_start(out=whh[:, G3 * k : G3 * (k + 1)],
                              in_=whh_d[128 * k : 128 * (k + 1), :])
        wx = pconst.tile([128, 4 * G3], F32R)
        for k in range(4):
            nc.sync.dma_start(out=wx[:, G3 * k : G3 * (k + 1)],
                              in_=wx_d[128 * k : 128 * (k + 1), :])
        bias_sb = pconst.tile([1, G3], F32R)
        nc.sync.dma_start(out=bias_sb, in_=bias_d[:, :])
        id128 = pconst.tile([128, 128], F32)
        nc.sync.dma_start(out=id128, in_=id128f_d[:, :])
        ones128 = pconst.tile([1, 128], F32R)
        nc.sync.dma_start(out=ones128, in_=ones_d[:, :])
        wfc = pconst.tile([128, KT * O], F32R)
        for k in range(KT):
            nc.sync.dma_start(out=wfc[:, O * k : O * (k + 1)],
                              in_=wfc_d[128 * k : 128 * (k + 1), :])
        bfc_sb = pconst.tile([1, O], F32R)
        nc.sync.dma_start(out=bfc_sb, in_=bfc_d[:, :])

        # persistent scan state (Q layout)
        hQ = pconst.tile([128, 2, 128], F32)
        nc.sync.dma_start(out=hQ, in_=h0q_d[:, :, :])
        hQbf = pconst.tile([128, 2, 128], BF16)
        nc.vector.tensor_copy(hQbf, hQ)

        # xpart staging tiles (ping-pong, 8 steps each, psum-scattered layout)
        xpsA = pconst.tile([128, 8, 3, 256], BF16)
        xpsB = pconst.tile([128, 8, 3, 256], BF16)
        nc.vector.memset(xpsA, 0.0)
        nc.vector.memset(xpsB, 0.0)

        # ---------------- precompute xpart ----------------
        ppre = ctx.enter_context(tc.tile_pool(name="ppre", bufs=2))
        ppre_ps = ctx.enter_context(tc.tile_pool(name="ppre_ps", bufs=2,
                                                 space="PSUM"))
        # t-major order: all t0=0 tiles are stored first, so their DRAM
        # writes complete long before the scan's first prefetch reads them
        # (DMA->DMA RAW through DRAM is not tracked by the tile framework).
        n_tb = nc_s // 128
        for rt in [b_ * n_tb + tb for tb in range(n_tb) for b_ in range(BL)]:
            r0 = rt * 128
            b = r0 // nc_s
            t0 = r0 % nc_s
            xt_sb = ppre.tile([128, 4, 128], F32R, tag="xt_sb")
            for k in range(4):
                nc.sync.dma_start(
                    out=xt_sb[:, k, :],
                    in_=xt_d[128 * k : 128 * (k + 1), r0 : r0 + 128])
            xp_sb = ppre.tile([128, G3], BF16, tag="xp_sb")
            for n in range(6):
                xp_ps = ppre_ps.tile([128, 512], F32, tag="xp_ps")
                ns = slice(512 * n, 512 * (n + 1))
                nc.tensor.matmul(xp_ps, ones128, bias_sb[:, ns],
                                 start=True, stop=False)
                for k in range(4):
                    nc.tensor.matmul(
                        xp_ps, xt_sb[:, k, :],
                        wx[:, G3 * k + 512 * n : G3 * k + 512 * (n + 1)],
                        start=False, stop=(k == 3))
                if n % 2 == 1:
                    nc.scalar.copy(xp_sb[:, ns], xp_ps)
                else:
                    nc.vector.tensor_copy(xp_sb[:, ns], xp_ps)
            nc.sync.dma_start(out=xpart_d[b, t0 : t0 + 128, :], in_=xp_sb)

        # ---------------- scan ----------------
        ptmp = ctx.enter_context(tc.tile_pool(name="ptmp", bufs=1))
        ppsum = ctx.enter_context(tc.tile_pool(name="ppsum", bufs=1, space="PSUM"))
        ptps = ctx.enter_context(tc.tile_pool(name="ptps", bufs=3, space="PSUM"))
        pools = (ptmp, ppsum, ptps)
        consts = (whh, id128, hQ, hQbf)

        def prefetch(dst, t_ap):
            # scatter [b, t, f] -> psum layout [32a+b, slot, g, c] on the read
            for a in range(4):
                for g in range(3):
                    fs = 1024 * g + 256 * a
                    nc.sync.dma_start(
                        out=dst[32 * a : 32 * a + 8, :, g, :],
                        in_=xpart_d[0:BL, t_ap, fs : fs + 256])

        prefetch(xpsA, ds(0, 8))
        with tc.For_i(0, nc_s, 16) as iv:
            prefetch(xpsB, ds(iv + 8, 8))
            for u in range(8):
                _scan_step(nc, tc, pools, consts, xpsA, u)
            prefetch(xpsA, ds(iv + 16, 8))
            for u in range(8):
                _scan_step(nc, tc, pools, consts, xpsB, u)

        # ---------------- FC head + log_softmax ----------------
        hrelu = ptmp.tile([128, 2, 128], F32R, tag="hrelu")
        nc.scalar.activation(hrelu, hQ, AFT.Relu)

        fc_ps = ptps.tile([BL, O], F32, tag="tp")
        nc.tensor.matmul(fc_ps, ones128[:, :BL], bfc_sb, start=True, stop=False)
        for k in range(KT):
            nc.tensor.matmul(fc_ps, _lhsT(hrelu, k),
                             wfc[:, O * k : O * (k + 1)],
                             start=False, stop=(k == KT - 1))

        mx = ptmp.tile([BL, 1], F32, tag="mx")
        nc.vector.tensor_reduce(mx, fc_ps, mybir.AxisListType.X,
                                mybir.AluOpType.max)
        tt = ptmp.tile([BL, O], F32, tag="tt")
        nc.vector.tensor_scalar(tt, fc_ps, mx, None, mybir.AluOpType.subtract)
        ex = ptmp.tile([BL, O], F32, tag="ex")
        nc.scalar.activation(ex, tt, AFT.Exp)
        sm = ptmp.tile([BL, 1], F32, tag="sm")
        nc.vector.tensor_reduce(sm, ex, mybir.AxisListType.X,
                                mybir.AluOpType.add)
        lsm = ptmp.tile([BL, 1], F32, tag="lsm")
        nc.scalar.activation(lsm, sm, AFT.Ln)
        res = ptmp.tile([BL, O], F32, tag="res")
        nc.vector.tensor_scalar(res, tt, lsm, None, mybir.AluOpType.subtract)
        nc.sync.dma_start(out=out_d[:, :], in_=res)

    nc.compile()
    return nc


def prep_inputs(x, h, Wz, bz, Wr, br, Wh, bh, Wfc, bfc, nc_s=S):
    """Host-side prep: shard + relayout. Returns per-core input maps."""
    f32 = np.float32
    x = np.asarray(x, f32)[:, :nc_s, :]
    h0 = np.asarray(h, f32)[:, 0, :]
    Wx_all = np.concatenate([np.asarray(Wr, f32)[:I], np.asarray(Wz, f32)[:I],
                             np.asarray(Wh, f32)[:I]], axis=1)
    b_all = np.concatenate([np.asarray(br, f32), np.asarray(bz, f32),
                            np.asarray(bh, f32)])[None, :]
    Whh_all = np.concatenate([np.asarray(Wr, f32)[I:], np.asarray(Wz, f32)[I:],
                              np.asarray(Wh, f32)[I:]], axis=1)
    Whh_bf = Whh_all.astype(ml_dtypes.bfloat16)
    id128 = np.eye(128, dtype=f32)
    id8 = np.zeros((8, 32), ml_dtypes.bfloat16)
    np.fill_diagonal(id8[:, :8], 1)
    wfc = np.asarray(Wfc, f32)
    bfc_a = np.asarray(bfc, f32)[None, :]

    in_maps = []
    for c in range(NCORES):
        xc = x[c * BL : (c + 1) * BL]                      # [8, S, I]
        xt = xc.reshape(BL * nc_s, I).T.copy()             # [I, 8*S]
        h0c = h0[c * BL : (c + 1) * BL]                    # [8, H]
        # Q layout: h0q[p, u, 32j+b] = h0c[b, 256j + 128u + p]
        h0q = np.zeros((128, 2, 128), f32)
        hv = h0c.reshape(BL, 4, 2, 128)                    # [b, j, u, p]
        for j in range(4):
            h0q[:, :, 32 * j : 32 * j + BL] = hv[:, j].transpose(2, 1, 0)
        in_maps.append({
            "xt": xt, "h0q": h0q,
            "wx": Wx_all, "bias": b_all, "whh": Whh_bf,
            "wfc": wfc, "bfc": bfc_a,
            "id128f": id128, "id8": id8, "ones": np.ones((1, 128), f32),
        })
    return in_maps


_BUILT = {}
_LAST_RESULTS = None


def kernel(**inputs):
    global _LAST_RESULTS
    key = "full"
    if key not in _BUILT:
        _BUILT[key] = build(S, unroll=8, use_for_i=True)
    nc = _BUILT[key]
    in_maps = prep_inputs(**inputs)
    trace = bool(int(os.environ.get("BASS_TRACE", "0") or "0"))
    res = run_bass_kernel_spmd(nc, in_maps, list(range(NCORES)), trace=trace)
    _LAST_RESULTS = res
    outs = [res.results[c]["out"] for c in range(NCORES)]
    return np.concatenate(outs, axis=0).astype(np.float32)


if __name__ == "__main__":
    np.random.seed(0)
    print("building...")
    nc = build(128, unroll=8, use_for_i=True)
    print("build ok:", nc)

